# revision 21
# baseline (speedup 1.0000x reference)
"""Distributed Trainium2 kernel for GNN message passing (COO SpMM + dense head).

out = relu((A @ x) @ W[:128] + x @ W[128:])   with A given as COO (rows, cols, vals)

Strategy (8 NeuronCores, SPMD single graph):
  - Rows (destinations) sharded across cores: core c owns rows [c*12500, (c+1)*12500).
  - Host-side preprocessing is index/layout-only: per core, edges are grouped by
    128-column slab of x; each slab's distinct destination rows become "tokens",
    and a selection matrix E_slab [128 cols x 128 tokens] (bf16, with adj_vals
    placed at the edge positions) is prebuilt so the TensorEngine computes all
    per-(row,slab) partial sums as  psum[token, feat] = E_slab^T @ x_slab  while
    x streams through SBUF sequentially.  This removes every gather descriptor
    (the Q7/SWDGE descriptor generator, ~7ns/descriptor, was the bottleneck).
  - Token partial sums are hardware scatter-added (SWDGE dma_scatter_add) into a
    parity-striped DRAM accumulator h[row*2 + stripe].  Within one scatter call
    (8 slabs = 1024 tokens) a row's 1st/2nd occurrence use stripes 0/1 and 3rd+
    occurrences are diverted to a small spill stream; calls are WAW-serialized,
    so duplicate destinations never race (the HW scatter-add is not atomic
    across DMA engines - measured).
  - The spill stream (token-capacity overflow + >2-per-call duplicates) uses the
    v2 machinery: col-chunked SWDGE gather + val scale + round/parity-dealt
    scatter.  It is a few hundred edges.
  - Dense head: per 128-row group, combine the two h stripes, h.T via PE
    transpose, out = relu(hT.T@W1 + xT.T@W2) accumulated in PSUM, relu on
    ScalarE, DMA out.
"""

import sys

if "/opt/trn_rl_repo" not in sys.path:
    sys.path.insert(0, "/opt/trn_rl_repo")

import numpy as np
import ml_dtypes

BF16 = np.dtype(ml_dtypes.bfloat16)

N_NODES = 100000
N_EDGES = 600000
D = 128
OUT = 128
P = 128
NCORES = 8
RPC = N_NODES // NCORES          # 12500 rows per core
NSLAB = 784                      # 128-col slabs (x padded to 100352 rows)
NPADROW = NSLAB * 128            # 100352
CAP = 128                        # tokens per slab (forced by scatter alignment)
GROUP = 8                        # slabs per scatter call (1024 tokens = ring cap)
NGROUPS = NSLAB // GROUP         # 98
H_PAD = 12800                    # padded row count per parity stripe
# spill stream constants (v2 machinery)
SP_NCHUNK = 4
SP_CHUNK = NPADROW // SP_NCHUNK  # 25088 (< 32768 for int16 gather idx)
SP_MAXROUNDS = 16
TILE_E = 1024                    # max edges per SWDGE call (descriptor ring)

_compiled = {}


def _wrap16(a):
    t = a.reshape(-1, 16).T
    return np.tile(t, (8, 1)).copy()


def _runs(key):
    """occurrence index of each element within runs of equal (sorted) key"""
    n = len(key)
    if n == 0:
        return np.zeros(0, np.int64), np.zeros(0, bool)
    change = np.empty(n, bool)
    change[0] = True
    change[1:] = key[1:] != key[:-1]
    starts = np.flatnonzero(change)
    occ = np.arange(n) - np.repeat(starts, np.diff(np.append(starts, n)))
    return occ, change


def _prep(adj_rows, adj_cols, adj_vals):
    rows = np.asarray(adj_rows).astype(np.int64)
    cols = np.asarray(adj_cols).astype(np.int64)
    vals = np.asarray(adj_vals).astype(np.float32)

    E_list, stok_list = [], []
    spill_per_core = []
    sp_sizes = np.zeros((NCORES, SP_NCHUNK, SP_MAXROUNDS), np.int64)

    for c in range(NCORES):
        m = (rows >= c * RPC) & (rows < (c + 1) * RPC)
        r = rows[m] - c * RPC
        co = cols[m]
        v = vals[m]
        slab = co // 128
        crel = co % 128
        o = np.lexsort((r, slab))
        r, co, v, slab, crel = r[o], co[o], v[o], slab[o], crel[o]

        # distinct (slab, row) pairs -> tokens
        key = slab * RPC + r
        _, new = _runs(key)
        pair_id = np.cumsum(new) - 1                  # per edge
        p_slab = slab[new]
        p_row = r[new]
        npairs = len(p_slab)
        # token rank j within slab
        occ_s, _ = _runs(p_slab)
        p_j = occ_s
        keep_pair = p_j < CAP
        # stripe: occurrence of row within its (group-of-8-slabs, row) set,
        # counted over kept pairs only
        p_grp = p_slab // GROUP
        okey = np.where(keep_pair, p_grp * RPC + p_row, -1)
        o2 = np.argsort(okey, kind="stable")
        occ_g = np.empty(npairs, np.int64)
        og, _ = _runs(okey[o2])
        occ_g[o2] = og
        p_stripe = occ_g
        keep_pair &= p_stripe < 2

        # edge-level keep mask
        keep_edge = keep_pair[pair_id]

        # build E [NSLAB, 128, CAP] f32 -> bf16
        E = np.zeros((NSLAB, 128, CAP), np.float32)
        ke = keep_edge
        np.add.at(E, (slab[ke], crel[ke], p_j[pair_id[ke]]), v[ke])
        E_list.append(np.ascontiguousarray(
            E.transpose(1, 0, 2).reshape(128, NSLAB * CAP)).astype(BF16))

        # scatter token idx (pads -> dump row past real data)
        stok = np.full(NSLAB * CAP, 25088, np.int16)
        kp = keep_pair
        stok[p_slab[kp] * CAP + p_j[kp]] = (p_row[kp] * 2 + p_stripe[kp]
                                            ).astype(np.int16)
        stok_list.append(_wrap16(stok))

        # ---- spill stream (v2 machinery) ----
        sm = ~keep_edge
        sr, sco, sv_ = r[sm], co[sm], v[sm]
        ch = sco // SP_CHUNK
        o3 = np.lexsort((sr, ch))
        sr, sco, sv_, ch = sr[o3], sco[o3], sv_[o3], ch[o3]
        skey = ch * RPC + sr
        socc, _ = _runs(skey)
        rnd = socc // 2
        assert rnd.max(initial=0) < SP_MAXROUNDS
        par = socc % 2
        o4 = np.lexsort((sr, rnd, ch))
        sr, sco, sv_, ch, rnd, par = (a[o4] for a in (sr, sco, sv_, ch, rnd, par))
        for k in range(SP_NCHUNK):
            mk = ch == k
            sp_sizes[c, k] = np.bincount(rnd[mk], minlength=SP_MAXROUNDS)
        spill_per_core.append((sr, sco % SP_CHUNK, sv_, ch, rnd, par))

    sp_caps = ((sp_sizes.max(axis=0) + 127) // 128) * 128
    Tsp = max(int(sp_caps.sum()), 128)

    sp_calls = []
    off = 0
    for k in range(SP_NCHUNK):
        for j in range(SP_MAXROUNDS):
            cap = int(sp_caps[k, j])
            for t0 in range(0, cap, TILE_E):
                sp_calls.append((k, off + t0, min(TILE_E, cap - t0)))
            off += cap

    gidx_w = np.zeros((NCORES, P, Tsp // 16), np.int16)
    sidx_w = np.zeros((NCORES, P, Tsp // 16), np.int16)
    sval_w = np.zeros((NCORES, P, Tsp // 128), np.float32)
    offs = np.concatenate([[0], np.cumsum(sp_caps.reshape(-1))]).astype(np.int64)
    for c in range(NCORES):
        sr, scoi, sv_, ch, rnd, par = spill_per_core[c]
        gi = np.zeros(Tsp, np.int16)
        si = np.full(Tsp, 25088, np.int16)
        sv2 = np.zeros(Tsp, np.float32)
        s = 0
        for k in range(SP_NCHUNK):
            for j in range(SP_MAXROUNDS):
                n = int(sp_sizes[c, k, j])
                d0 = int(offs[k * SP_MAXROUNDS + j])
                gi[d0:d0 + n] = scoi[s:s + n]
                si[d0:d0 + n] = sr[s:s + n] * 2 + par[s:s + n]
                sv2[d0:d0 + n] = sv_[s:s + n]
                s += n
        gidx_w[c] = _wrap16(gi)
        sidx_w[c] = _wrap16(si)
        sval_w[c] = sv2.reshape(-1, 128).T

    return tuple(sp_calls), E_list, stok_list, gidx_w, sidx_w, sval_w


def _build(sp_calls):
    from concourse import bass, mybir, tile, bacc
    from concourse.masks import make_identity

    f32 = mybir.dt.float32
    bf16 = mybir.dt.bfloat16
    i16 = mybir.dt.int16
    Tsp = max(e0 + n for _, e0, n in sp_calls)

    nc = bacc.Bacc("TRN2", target_bir_lowering=False, debug=False,
                   num_swdge_queues=4)

    xs_d = nc.dram_tensor("xslab", [NPADROW, D], bf16, kind="ExternalInput")
    e_d = nc.dram_tensor("E", [P, NSLAB * CAP], bf16, kind="ExternalInput")
    stok_d = nc.dram_tensor("stok", [P, NSLAB * CAP // 16], i16,
                            kind="ExternalInput")
    xT_d = nc.dram_tensor("xlocT", [D, RPC], f32, kind="ExternalInput")
    w_d = nc.dram_tensor("W", [2 * D, OUT], f32, kind="ExternalInput")
    gidx_d = nc.dram_tensor("gidx", [P, Tsp // 16], i16, kind="ExternalInput")
    sidx_d = nc.dram_tensor("sidx", [P, Tsp // 16], i16, kind="ExternalInput")
    sval_d = nc.dram_tensor("svals", [P, Tsp // 128], f32, kind="ExternalInput")
    out_d = nc.dram_tensor("out", [RPC, OUT], f32, kind="ExternalOutput")
    h_d = nc.dram_tensor("h_acc", [2 * H_PAD, D], f32)   # parity-striped

    relu = mybir.ActivationFunctionType.Relu

    with tile.TileContext(nc) as tc:
        with tc.tile_pool(name="const", bufs=1) as constp, \
             tc.tile_pool(name="mess", bufs=3) as messp, \
             tc.tile_pool(name="meta", bufs=3) as metap, \
             tc.tile_pool(name="dense", bufs=4) as densep, \
             tc.tile_pool(name="psum", bufs=4, space="PSUM") as psump, \
             tc.tile_pool(name="psumd", bufs=2, space="PSUM") as psumd:

            ident = constp.tile([P, P], f32)
            make_identity(nc, ident[:])
            w1 = constp.tile([D, OUT], f32)
            nc.sync.dma_start(out=w1[:], in_=w_d[:D, :])
            w2 = constp.tile([D, OUT], f32)
            nc.sync.dma_start(out=w2[:], in_=w_d[D:, :])

            # zero the striped h accumulator (2*H_PAD = 25600 rows)
            zblk = constp.tile([P, 8, D], f32)
            nc.vector.memset(zblk[:], 0.0)
            for b in range(2 * H_PAD // 1024):
                dst = h_d[b * 1024:(b + 1) * 1024, :].rearrange(
                    "(a p) d -> p a d", p=P)
                nc.scalar.dma_start(out=dst, in_=zblk[:])

            # ---- SpMM main: E-matmul per slab, scatter-add per 8-slab group
            for g in range(NGROUPS):
                xt = messp.tile([P, GROUP, D], bf16, tag="xt")
                nc.sync.dma_start(
                    out=xt[:],
                    in_=xs_d[g * 1024:(g + 1) * 1024, :].rearrange(
                        "(a p) d -> p a d", p=P))
                et = messp.tile([P, GROUP * CAP], bf16, tag="et")
                nc.scalar.dma_start(
                    out=et[:],
                    in_=e_d[:, g * GROUP * CAP:(g + 1) * GROUP * CAP])
                st = messp.tile([P, GROUP, D], f32, tag="st")
                for s in range(GROUP):
                    pm = psump.tile([P, D], f32, tag="pm")
                    nc.tensor.matmul(pm[:], et[:, s * CAP:(s + 1) * CAP],
                                     xt[:, s, :], start=True, stop=True)
                    if s % 2 == 0:
                        nc.vector.tensor_copy(st[:, s, :], pm[:])
                    else:
                        nc.scalar.activation(
                            st[:, s, :], pm[:],
                            mybir.ActivationFunctionType.Copy)
                si = metap.tile([P, GROUP * CAP // 16], i16, tag="si")
                nc.sync.dma_start(
                    out=si[:],
                    in_=stok_d[:, g * 64:(g + 1) * 64])
                nc.gpsimd.dma_scatter_add(
                    h_d[:], st[:], si[:], GROUP * CAP, GROUP * CAP, D)

            # ---- spill stream (v2 machinery) ----
            for k, e0, n in sp_calls:
                ns = n // 128
                x_chunk = xs_d[k * SP_CHUNK:(k + 1) * SP_CHUNK, :]
                gi = metap.tile([P, TILE_E // 16], i16, tag="gi")
                nc.sync.dma_start(
                    out=gi[:, :n // 16],
                    in_=gidx_d[:, e0 // 16:(e0 + n) // 16])
                mvb = messp.tile([P, TILE_E // 128, D], bf16, tag="mvb")
                nc.gpsimd.dma_gather(
                    mvb[:, :ns, :], x_chunk, gi[:, :n // 16], n, n, D)
                sv = metap.tile([P, TILE_E // 128], f32, tag="sv")
                nc.sync.dma_start(
                    out=sv[:, :ns],
                    in_=sval_d[:, e0 // 128:(e0 + n) // 128])
                mv = messp.tile([P, TILE_E // 128, D], f32, tag="mv")
                nc.vector.tensor_tensor(
                    out=mv[:, :ns, :], in0=mvb[:, :ns, :],
                    in1=sv[:, :ns, None].to_broadcast([P, ns, D]),
                    op=mybir.AluOpType.mult)
                si = metap.tile([P, TILE_E // 16], i16, tag="si2")
                nc.scalar.dma_start(
                    out=si[:, :n // 16],
                    in_=sidx_d[:, e0 // 16:(e0 + n) // 16])
                nc.gpsimd.dma_scatter_add(
                    h_d[:], mv[:, :ns, :], si[:, :n // 16], n, n, D)

            # ---- dense head: out = relu(h @ W1 + x @ W2) ----
            h_pairs = h_d[:].rearrange("(a two) d -> a two d", two=2)
            ngroups = (RPC + P - 1) // P       # 98 (97 full + one 84-row tail)
            for g in range(ngroups):
                g0 = g * P
                rsz = min(P, RPC - g0)
                hb = densep.tile([P, D], f32, tag="hb")
                nc.sync.dma_start(out=hb[:rsz, :],
                                  in_=h_pairs[g0:g0 + rsz, 0, :])
                hc = densep.tile([P, D], f32, tag="hc")
                nc.sync.dma_start(out=hc[:rsz, :],
                                  in_=h_pairs[g0:g0 + rsz, 1, :])
                nc.vector.tensor_add(out=hb[:rsz, :], in0=hb[:rsz, :],
                                     in1=hc[:rsz, :])
                pt = psumd.tile([P, P], f32, tag="pt")
                nc.tensor.transpose(pt[:, :rsz], hb[:rsz, :], ident[:rsz, :rsz])
                hT = densep.tile([P, P], f32, tag="hT")
                nc.vector.tensor_copy(hT[:, :rsz], pt[:, :rsz])
                xT = densep.tile([P, P], f32, tag="xT")
                nc.sync.dma_start(out=xT[:, :rsz], in_=xT_d[:, g0:g0 + rsz])
                po = psumd.tile([P, OUT], f32, tag="po")
                nc.tensor.matmul(po[:rsz, :], hT[:, :rsz], w1[:],
                                 start=True, stop=False)
                nc.tensor.matmul(po[:rsz, :], xT[:, :rsz], w2[:],
                                 start=False, stop=True)
                ob = densep.tile([P, OUT], f32, tag="ob")
                nc.scalar.activation(ob[:rsz, :], po[:rsz, :], relu)
                nc.scalar.dma_start(out=out_d[g0:g0 + rsz, :], in_=ob[:rsz, :])

    nc.compile()
    return nc


def _get_nc(sp_calls):
    nc = _compiled.get(sp_calls)
    if nc is None:
        nc = _build(sp_calls)
        _compiled[sp_calls] = nc
    return nc


def _make_in_maps(x, W, prep):
    sp_calls, E_list, stok_list, gidx_w, sidx_w, sval_w = prep
    x = np.ascontiguousarray(np.asarray(x, np.float32))
    W = np.ascontiguousarray(np.asarray(W, np.float32))
    xpad = np.zeros((NPADROW, D), np.float32)
    xpad[:N_NODES] = x
    x16 = xpad.astype(BF16)
    in_maps = []
    for c in range(NCORES):
        xloc = x[c * RPC:(c + 1) * RPC]
        in_maps.append({
            "xslab": x16,
            "E": E_list[c],
            "stok": stok_list[c],
            "xlocT": np.ascontiguousarray(xloc.T),
            "W": W,
            "gidx": gidx_w[c],
            "sidx": sidx_w[c],
            "svals": sval_w[c],
        })
    return in_maps


def _install_trace_shims():
    """Make trace=True work in this container: provide antenv.axon_hooks
    (ctypes NTFF profiling via the axon PJRT .so) and stub the artifact
    upload (no bucket access here)."""
    import contextlib
    import ctypes
    import types

    try:
        import antenv.axon_hooks  # noqa: F401
        has_hooks = True
    except ImportError:
        has_hooks = False
    if not has_hooks:
        so_path = "/opt/axon/libaxon_pjrt.so"
        lib = ctypes.CDLL(so_path)
        if hasattr(lib, "axon_start_nrt_profile"):
            lib.axon_start_nrt_profile.argtypes = [
                ctypes.POINTER(ctypes.c_int64), ctypes.c_size_t]
            lib.axon_start_nrt_profile.restype = ctypes.c_int64
            lib.axon_stop_nrt_profile.argtypes = [ctypes.c_char_p]
            lib.axon_stop_nrt_profile.restype = ctypes.c_int64

            @contextlib.contextmanager
            def _hook(output_dir, device_ids):
                import jax
                jax.devices()
                if device_ids:
                    ids = (ctypes.c_int64 * len(device_ids))(*device_ids)
                    rc = lib.axon_start_nrt_profile(ids, len(device_ids))
                else:
                    rc = lib.axon_start_nrt_profile(None, 0)
                if rc != 0:
                    raise RuntimeError(f"axon_start_nrt_profile rc={rc}")
                try:
                    yield
                finally:
                    n = lib.axon_stop_nrt_profile(str(output_dir).encode())
                    if n <= 0:
                        print(f"ntff profile: rc={n} (no files?) at {output_dir}")

            mod = types.ModuleType("antenv.axon_hooks")
            mod.get_axon_ntff_profile_hook = lambda: _hook
            mod.set_axon_ntff_profile_hook = lambda h: None
            sys.modules["antenv.axon_hooks"] = mod

    import concourse.bass_utils as bu
    bu.upload_artifacts = lambda tmpdir: f"local:{tmpdir}"


def _run(x, adj_rows, adj_cols, adj_vals, W, trace=False):
    from concourse.bass_utils import run_bass_kernel_spmd
    if trace:
        try:
            _install_trace_shims()
        except Exception as e:  # tracing is best-effort
            print("trace shim install failed:", e)
    prep = _prep(adj_rows, adj_cols, adj_vals)
    nc = _get_nc(prep[0])
    in_maps = _make_in_maps(x, W, prep)
    res = run_bass_kernel_spmd(nc, in_maps, list(range(NCORES)), trace=trace)
    out = np.concatenate([res.results[c]["out"] for c in range(NCORES)], axis=0)
    return out, res


def kernel(x, adj_rows, adj_cols, adj_vals, W):
    out, _ = _run(x, adj_rows, adj_cols, adj_vals, W, trace=False)
    return out


# revision 22
# speedup vs baseline: 1.4623x; 1.4623x over previous
"""Distributed Trainium2 kernel for GNN message passing (COO SpMM + dense head).

out = relu((A @ x) @ W[:128] + x @ W[128:])   with A given as COO (rows, cols, vals)

Strategy (8 NeuronCores, SPMD single graph):
  - Rows (destinations) sharded across cores: core c owns rows [c*12500, (c+1)*12500).
  - Host-side preprocessing is index/layout-only: per core, edges are grouped by
    128-column slab of x; each slab's distinct destination rows become "tokens",
    and a selection matrix E_slab [128 cols x 128 tokens] (bf16, with adj_vals
    placed at the edge positions) is prebuilt so the TensorEngine computes all
    per-(row,slab) partial sums as  psum[token, feat] = E_slab^T @ x_slab  while
    x streams through SBUF sequentially.  This removes every gather descriptor
    (the Q7/SWDGE descriptor generator, ~7ns/descriptor, was the bottleneck).
  - Token partial sums are hardware scatter-added (SWDGE dma_scatter_add) into a
    parity-striped DRAM accumulator h[row*2 + stripe].  Within one scatter call
    (8 slabs = 1024 tokens) a row's 1st/2nd occurrence use stripes 0/1 and 3rd+
    occurrences are diverted to a small spill stream; calls are WAW-serialized,
    so duplicate destinations never race (the HW scatter-add is not atomic
    across DMA engines - measured).
  - The spill stream (token-capacity overflow + >2-per-call duplicates) uses the
    v2 machinery: col-chunked SWDGE gather + val scale + round/parity-dealt
    scatter.  It is a few hundred edges.
  - Dense head: per 128-row group, combine the two h stripes, h.T via PE
    transpose, out = relu(hT.T@W1 + xT.T@W2) accumulated in PSUM, relu on
    ScalarE, DMA out.
"""

import sys

if "/opt/trn_rl_repo" not in sys.path:
    sys.path.insert(0, "/opt/trn_rl_repo")

import numpy as np
import ml_dtypes

BF16 = np.dtype(ml_dtypes.bfloat16)

N_NODES = 100000
N_EDGES = 600000
D = 128
OUT = 128
P = 128
NCORES = 8
RPC = N_NODES // NCORES          # 12500 rows per core
NSLAB = 784                      # 128-col slabs (x padded to 100352 rows)
NPADROW = NSLAB * 128            # 100352
CAP = 128                        # tokens per slab (forced by scatter alignment)
GROUP = 8                        # slabs per scatter call (1024 tokens = ring cap)
NGROUPS = NSLAB // GROUP         # 98
H_PAD = 12800                    # padded row count per parity stripe
# spill stream constants (v2 machinery)
SP_NCHUNK = 4
SP_CHUNK = NPADROW // SP_NCHUNK  # 25088 (< 32768 for int16 gather idx)
SP_MAXROUNDS = 16
TILE_E = 1024                    # max edges per SWDGE call (descriptor ring)

_compiled = {}


def _wrap16(a):
    t = a.reshape(-1, 16).T
    return np.tile(t, (8, 1)).copy()


def _runs(key):
    """occurrence index of each element within runs of equal (sorted) key"""
    n = len(key)
    if n == 0:
        return np.zeros(0, np.int64), np.zeros(0, bool)
    change = np.empty(n, bool)
    change[0] = True
    change[1:] = key[1:] != key[:-1]
    starts = np.flatnonzero(change)
    occ = np.arange(n) - np.repeat(starts, np.diff(np.append(starts, n)))
    return occ, change


def _prep(adj_rows, adj_cols, adj_vals):
    rows = np.asarray(adj_rows).astype(np.int64)
    cols = np.asarray(adj_cols).astype(np.int64)
    vals = np.asarray(adj_vals).astype(np.float32)

    E_list, stok_list = [], []
    spill_per_core = []
    sp_sizes = np.zeros((NCORES, SP_NCHUNK, SP_MAXROUNDS), np.int64)

    for c in range(NCORES):
        m = (rows >= c * RPC) & (rows < (c + 1) * RPC)
        r = rows[m] - c * RPC
        co = cols[m]
        v = vals[m]
        slab = co // 128
        crel = co % 128
        o = np.lexsort((r, slab))
        r, co, v, slab, crel = r[o], co[o], v[o], slab[o], crel[o]

        # distinct (slab, row) pairs -> tokens
        key = slab * RPC + r
        _, new = _runs(key)
        pair_id = np.cumsum(new) - 1                  # per edge
        p_slab = slab[new]
        p_row = r[new]
        npairs = len(p_slab)
        # token rank j within slab
        occ_s, _ = _runs(p_slab)
        p_j = occ_s
        keep_pair = p_j < CAP
        # stripe: occurrence of row within its (group-of-8-slabs, row) set,
        # counted over kept pairs only
        p_grp = p_slab // GROUP
        okey = np.where(keep_pair, p_grp * RPC + p_row, -1)
        o2 = np.argsort(okey, kind="stable")
        occ_g = np.empty(npairs, np.int64)
        og, _ = _runs(okey[o2])
        occ_g[o2] = og
        p_stripe = occ_g
        keep_pair &= p_stripe < 2

        # edge-level keep mask
        keep_edge = keep_pair[pair_id]

        # build E [NSLAB, 128, CAP] f32 -> bf16
        E = np.zeros((NSLAB, 128, CAP), np.float32)
        ke = keep_edge
        np.add.at(E, (slab[ke], crel[ke], p_j[pair_id[ke]]), v[ke])
        E_list.append(np.ascontiguousarray(
            E.transpose(1, 0, 2).reshape(128, NSLAB * CAP)).astype(BF16))

        # scatter token idx (pads -> dump row past real data)
        stok = np.full(NSLAB * CAP, 25088, np.int16)
        kp = keep_pair
        stok[p_slab[kp] * CAP + p_j[kp]] = (p_row[kp] * 2 + p_stripe[kp]
                                            ).astype(np.int16)
        stok_list.append(_wrap16(stok))

        # ---- spill stream (v2 machinery) ----
        sm = ~keep_edge
        sr, sco, sv_ = r[sm], co[sm], v[sm]
        ch = sco // SP_CHUNK
        o3 = np.lexsort((sr, ch))
        sr, sco, sv_, ch = sr[o3], sco[o3], sv_[o3], ch[o3]
        skey = ch * RPC + sr
        socc, _ = _runs(skey)
        rnd = socc // 2
        assert rnd.max(initial=0) < SP_MAXROUNDS
        par = socc % 2
        o4 = np.lexsort((sr, rnd, ch))
        sr, sco, sv_, ch, rnd, par = (a[o4] for a in (sr, sco, sv_, ch, rnd, par))
        for k in range(SP_NCHUNK):
            mk = ch == k
            sp_sizes[c, k] = np.bincount(rnd[mk], minlength=SP_MAXROUNDS)
        spill_per_core.append((sr, sco % SP_CHUNK, sv_, ch, rnd, par))

    sp_caps = ((sp_sizes.max(axis=0) + 127) // 128) * 128
    Tsp = max(int(sp_caps.sum()), 128)

    sp_calls = []
    off = 0
    for k in range(SP_NCHUNK):
        for j in range(SP_MAXROUNDS):
            cap = int(sp_caps[k, j])
            for t0 in range(0, cap, TILE_E):
                sp_calls.append((k, off + t0, min(TILE_E, cap - t0)))
            off += cap

    gidx_w = np.zeros((NCORES, P, Tsp // 16), np.int16)
    sidx_w = np.zeros((NCORES, P, Tsp // 16), np.int16)
    sval_w = np.zeros((NCORES, P, Tsp // 128), np.float32)
    offs = np.concatenate([[0], np.cumsum(sp_caps.reshape(-1))]).astype(np.int64)
    for c in range(NCORES):
        sr, scoi, sv_, ch, rnd, par = spill_per_core[c]
        gi = np.zeros(Tsp, np.int16)
        si = np.full(Tsp, 25088, np.int16)
        sv2 = np.zeros(Tsp, np.float32)
        s = 0
        for k in range(SP_NCHUNK):
            for j in range(SP_MAXROUNDS):
                n = int(sp_sizes[c, k, j])
                d0 = int(offs[k * SP_MAXROUNDS + j])
                gi[d0:d0 + n] = scoi[s:s + n]
                si[d0:d0 + n] = sr[s:s + n] * 2 + par[s:s + n]
                sv2[d0:d0 + n] = sv_[s:s + n]
                s += n
        gidx_w[c] = _wrap16(gi)
        sidx_w[c] = _wrap16(si)
        sval_w[c] = sv2.reshape(-1, 128).T

    return tuple(sp_calls), E_list, stok_list, gidx_w, sidx_w, sval_w


def _build(sp_calls):
    from concourse import bass, mybir, tile, bacc
    from concourse.masks import make_identity

    f32 = mybir.dt.float32
    bf16 = mybir.dt.bfloat16
    i16 = mybir.dt.int16
    Tsp = max(e0 + n for _, e0, n in sp_calls)

    nc = bacc.Bacc("TRN2", target_bir_lowering=False, debug=False,
                   num_swdge_queues=4)

    xs_d = nc.dram_tensor("xslab", [P, NPADROW], bf16, kind="ExternalInput")
    xr_d = nc.dram_tensor("xrow", [NPADROW, D], bf16, kind="ExternalInput")
    e_d = nc.dram_tensor("E", [P, NSLAB * CAP], bf16, kind="ExternalInput")
    stok_d = nc.dram_tensor("stok", [P, NSLAB * CAP // 16], i16,
                            kind="ExternalInput")
    xT_d = nc.dram_tensor("xlocT", [D, RPC], f32, kind="ExternalInput")
    w_d = nc.dram_tensor("W", [2 * D, OUT], f32, kind="ExternalInput")
    gidx_d = nc.dram_tensor("gidx", [P, Tsp // 16], i16, kind="ExternalInput")
    sidx_d = nc.dram_tensor("sidx", [P, Tsp // 16], i16, kind="ExternalInput")
    sval_d = nc.dram_tensor("svals", [P, Tsp // 128], f32, kind="ExternalInput")
    out_d = nc.dram_tensor("out", [RPC, OUT], f32, kind="ExternalOutput")
    h_d = nc.dram_tensor("h_acc", [2 * H_PAD, D], f32)   # parity-striped

    relu = mybir.ActivationFunctionType.Relu

    with tile.TileContext(nc) as tc:
        with tc.tile_pool(name="const", bufs=1) as constp, \
             tc.tile_pool(name="mess", bufs=3) as messp, \
             tc.tile_pool(name="meta", bufs=3) as metap, \
             tc.tile_pool(name="dense", bufs=4) as densep, \
             tc.tile_pool(name="psum", bufs=4, space="PSUM") as psump, \
             tc.tile_pool(name="psumd", bufs=2, space="PSUM") as psumd:

            ident = constp.tile([P, P], f32)
            make_identity(nc, ident[:])
            w1 = constp.tile([D, OUT], f32)
            nc.sync.dma_start(out=w1[:], in_=w_d[:D, :])
            w2 = constp.tile([D, OUT], f32)
            nc.sync.dma_start(out=w2[:], in_=w_d[D:, :])

            # zero the striped h accumulator (2*H_PAD = 25600 rows)
            zblk = constp.tile([P, 8, D], f32)
            nc.vector.memset(zblk[:], 0.0)
            for b in range(2 * H_PAD // 1024):
                dst = h_d[b * 1024:(b + 1) * 1024, :].rearrange(
                    "(a p) d -> p a d", p=P)
                nc.scalar.dma_start(out=dst, in_=zblk[:])

            # ---- SpMM main: E-matmul per slab, scatter-add per 8-slab group
            for g in range(NGROUPS):
                xt = messp.tile([P, GROUP, D], bf16, tag="xt")
                nc.sync.dma_start(
                    out=xt[:],
                    in_=xs_d[:, g * 1024:(g + 1) * 1024].rearrange(
                        "p (a d) -> p a d", d=D))
                et = messp.tile([P, GROUP * CAP], bf16, tag="et")
                nc.scalar.dma_start(
                    out=et[:],
                    in_=e_d[:, g * GROUP * CAP:(g + 1) * GROUP * CAP])
                st = messp.tile([P, GROUP, D], f32, tag="st")
                for s in range(GROUP):
                    pm = psump.tile([P, D], f32, tag="pm")
                    nc.tensor.matmul(pm[:], et[:, s * CAP:(s + 1) * CAP],
                                     xt[:, s, :], start=True, stop=True)
                    if s % 2 == 0:
                        nc.vector.tensor_copy(st[:, s, :], pm[:])
                    else:
                        nc.scalar.activation(
                            st[:, s, :], pm[:],
                            mybir.ActivationFunctionType.Copy)
                si = metap.tile([P, GROUP * CAP // 16], i16, tag="si")
                nc.sync.dma_start(
                    out=si[:],
                    in_=stok_d[:, g * 64:(g + 1) * 64])
                nc.gpsimd.dma_scatter_add(
                    h_d[:], st[:], si[:], GROUP * CAP, GROUP * CAP, D)

            # ---- spill stream (v2 machinery) ----
            for k, e0, n in sp_calls:
                ns = n // 128
                x_chunk = xr_d[k * SP_CHUNK:(k + 1) * SP_CHUNK, :]
                gi = metap.tile([P, TILE_E // 16], i16, tag="gi")
                nc.sync.dma_start(
                    out=gi[:, :n // 16],
                    in_=gidx_d[:, e0 // 16:(e0 + n) // 16])
                mvb = messp.tile([P, TILE_E // 128, D], bf16, tag="mvb")
                nc.gpsimd.dma_gather(
                    mvb[:, :ns, :], x_chunk, gi[:, :n // 16], n, n, D)
                sv = metap.tile([P, TILE_E // 128], f32, tag="sv")
                nc.sync.dma_start(
                    out=sv[:, :ns],
                    in_=sval_d[:, e0 // 128:(e0 + n) // 128])
                mv = messp.tile([P, TILE_E // 128, D], f32, tag="mv")
                nc.vector.tensor_tensor(
                    out=mv[:, :ns, :], in0=mvb[:, :ns, :],
                    in1=sv[:, :ns, None].to_broadcast([P, ns, D]),
                    op=mybir.AluOpType.mult)
                si = metap.tile([P, TILE_E // 16], i16, tag="si2")
                nc.scalar.dma_start(
                    out=si[:, :n // 16],
                    in_=sidx_d[:, e0 // 16:(e0 + n) // 16])
                nc.gpsimd.dma_scatter_add(
                    h_d[:], mv[:, :ns, :], si[:, :n // 16], n, n, D)

            # ---- dense head: out = relu(h @ W1 + x @ W2) ----
            h_pairs = h_d[:].rearrange("(a two) d -> a two d", two=2)
            ngroups = (RPC + P - 1) // P       # 98 (97 full + one 84-row tail)
            for g in range(ngroups):
                g0 = g * P
                rsz = min(P, RPC - g0)
                hb = densep.tile([P, D], f32, tag="hb")
                nc.sync.dma_start(out=hb[:rsz, :],
                                  in_=h_pairs[g0:g0 + rsz, 0, :])
                hc = densep.tile([P, D], f32, tag="hc")
                nc.sync.dma_start(out=hc[:rsz, :],
                                  in_=h_pairs[g0:g0 + rsz, 1, :])
                nc.vector.tensor_add(out=hb[:rsz, :], in0=hb[:rsz, :],
                                     in1=hc[:rsz, :])
                pt = psumd.tile([P, P], f32, tag="pt")
                nc.tensor.transpose(pt[:, :rsz], hb[:rsz, :], ident[:rsz, :rsz])
                hT = densep.tile([P, P], f32, tag="hT")
                nc.vector.tensor_copy(hT[:, :rsz], pt[:, :rsz])
                xT = densep.tile([P, P], f32, tag="xT")
                nc.sync.dma_start(out=xT[:, :rsz], in_=xT_d[:, g0:g0 + rsz])
                po = psumd.tile([P, OUT], f32, tag="po")
                nc.tensor.matmul(po[:rsz, :], hT[:, :rsz], w1[:],
                                 start=True, stop=False)
                nc.tensor.matmul(po[:rsz, :], xT[:, :rsz], w2[:],
                                 start=False, stop=True)
                ob = densep.tile([P, OUT], f32, tag="ob")
                nc.scalar.activation(ob[:rsz, :], po[:rsz, :], relu)
                nc.scalar.dma_start(out=out_d[g0:g0 + rsz, :], in_=ob[:rsz, :])

    nc.compile()
    return nc


def _get_nc(sp_calls):
    nc = _compiled.get(sp_calls)
    if nc is None:
        nc = _build(sp_calls)
        _compiled[sp_calls] = nc
    return nc


def _make_in_maps(x, W, prep):
    sp_calls, E_list, stok_list, gidx_w, sidx_w, sval_w = prep
    x = np.ascontiguousarray(np.asarray(x, np.float32))
    W = np.ascontiguousarray(np.asarray(W, np.float32))
    xpad = np.zeros((NPADROW, D), np.float32)
    xpad[:N_NODES] = x
    x16 = xpad.astype(BF16)
    xshuf = np.ascontiguousarray(
        x16.reshape(NSLAB, 128, D).transpose(1, 0, 2).reshape(P, NSLAB * D))
    in_maps = []
    for c in range(NCORES):
        xloc = x[c * RPC:(c + 1) * RPC]
        in_maps.append({
            "xslab": xshuf,
            "xrow": x16,
            "E": E_list[c],
            "stok": stok_list[c],
            "xlocT": np.ascontiguousarray(xloc.T),
            "W": W,
            "gidx": gidx_w[c],
            "sidx": sidx_w[c],
            "svals": sval_w[c],
        })
    return in_maps


def _install_trace_shims():
    """Make trace=True work in this container: provide antenv.axon_hooks
    (ctypes NTFF profiling via the axon PJRT .so) and stub the artifact
    upload (no bucket access here)."""
    import contextlib
    import ctypes
    import types

    try:
        import antenv.axon_hooks  # noqa: F401
        has_hooks = True
    except ImportError:
        has_hooks = False
    if not has_hooks:
        so_path = "/opt/axon/libaxon_pjrt.so"
        lib = ctypes.CDLL(so_path)
        if hasattr(lib, "axon_start_nrt_profile"):
            lib.axon_start_nrt_profile.argtypes = [
                ctypes.POINTER(ctypes.c_int64), ctypes.c_size_t]
            lib.axon_start_nrt_profile.restype = ctypes.c_int64
            lib.axon_stop_nrt_profile.argtypes = [ctypes.c_char_p]
            lib.axon_stop_nrt_profile.restype = ctypes.c_int64

            @contextlib.contextmanager
            def _hook(output_dir, device_ids):
                import jax
                jax.devices()
                if device_ids:
                    ids = (ctypes.c_int64 * len(device_ids))(*device_ids)
                    rc = lib.axon_start_nrt_profile(ids, len(device_ids))
                else:
                    rc = lib.axon_start_nrt_profile(None, 0)
                if rc != 0:
                    raise RuntimeError(f"axon_start_nrt_profile rc={rc}")
                try:
                    yield
                finally:
                    n = lib.axon_stop_nrt_profile(str(output_dir).encode())
                    if n <= 0:
                        print(f"ntff profile: rc={n} (no files?) at {output_dir}")

            mod = types.ModuleType("antenv.axon_hooks")
            mod.get_axon_ntff_profile_hook = lambda: _hook
            mod.set_axon_ntff_profile_hook = lambda h: None
            sys.modules["antenv.axon_hooks"] = mod

    import concourse.bass_utils as bu
    bu.upload_artifacts = lambda tmpdir: f"local:{tmpdir}"


def _run(x, adj_rows, adj_cols, adj_vals, W, trace=False):
    from concourse.bass_utils import run_bass_kernel_spmd
    if trace:
        try:
            _install_trace_shims()
        except Exception as e:  # tracing is best-effort
            print("trace shim install failed:", e)
    prep = _prep(adj_rows, adj_cols, adj_vals)
    nc = _get_nc(prep[0])
    in_maps = _make_in_maps(x, W, prep)
    res = run_bass_kernel_spmd(nc, in_maps, list(range(NCORES)), trace=trace)
    out = np.concatenate([res.results[c]["out"] for c in range(NCORES)], axis=0)
    return out, res


def kernel(x, adj_rows, adj_cols, adj_vals, W):
    out, _ = _run(x, adj_rows, adj_cols, adj_vals, W, trace=False)
    return out
